# revision 30
# baseline (speedup 1.0000x reference)
"""Trainium2 Bass kernel for the GODEFunc graph-ODE message-passing module.

Math (per batch b):
    xa   = sum_k conv_w[k] * (adj[k] @ x[b]) + conv_b
    W    = (w * clip(d,0,1)) @ w.T
    out  = tanh(0.5*sigmoid(alpha) * xa - 2*x[b] + x[b] @ W + x0[b]*sigmoid(beta))

Sharding: rows (nodes) split across 8 cores; each core computes its
1024-row slice of the output for all batches.  No collectives needed.

Key layout decisions (all host-side staging, no host arithmetic):
  - adj is passed per-core TRANSPOSED (adjT[k][m, i] = adj[k, row_i, m]) so
    the PE reads contraction-major lhsT tiles straight from SBUF -- no PE
    transposes, no PSUM->SBUF copies.
  - x / x0 / x^T are passed pre-permuted so every DMA reads >=4KB
    contiguous per partition (the natural layouts produce 128-256B
    descriptors which run far below DMA line rate).
  - adj k=0 streams on the Sync HWDGE queue, k=1 on the Scalar HWDGE
    queue, x-family on the GpSimd SWDGE queue (casting x/x0 to bf16
    in-DMA): three parallel DMA queues; the two HWDGE queues together
    saturate the ~358 GB/s per-core HBM limit.
  - The Sync/Scalar engines carry ONLY DMA triggers during the main loop:
    an HWDGE dma_start waits at the issuing engine's sequencer, so any
    compute op between triggers serializes the queue with compute.

Per-core kernel structure:
  - DVE combines the two fp32 adjT streams into one bf16 a_eff tile
    (conv_w folded in, in-place, group-batched); the 0.5*sigmoid(alpha)
    row scale is applied at the epilogue where rows are the partition dim.
  - Main matmuls accumulate psum_y[ntt] += a_effT_slice.T @ x4[mc] over
    all 64 contraction chunks (3-group-deep DMA prefetch; the last group's
    DMAs and combines are per-chunk to shorten the post-last-byte tail).
  - x @ (W - 2I) accumulates into a second PSUM set via host-provided x^T
    (fp32: this is the dominant output term, bf16 here costs ~1e-2 error).
  - x0*sig(beta) + psum_xw is folded into one SBUF tile mid-kernel, so the
    tail epilogue is one DVE op + tanh per row tile; y writes go out on
    the by-then-idle HWDGE queues.
"""

import sys

for _p in ("/opt/trn_rl_repo",):
    if _p not in sys.path:
        sys.path.insert(0, _p)

from contextlib import ExitStack

import numpy as np

import concourse.bass as bass
import concourse.mybir as mybir
import concourse.tile as tile
from concourse import bacc
from concourse.bass_utils import run_bass_kernel_spmd
from concourse.masks import make_identity

dt = mybir.dt
AF = mybir.ActivationFunctionType
ALU = mybir.AluOpType

B, N, F, K = 4, 8192, 64, 2
N_CORES = 8
P = 128
GRP = 4  # contraction chunks per adj DMA group


def build_kernel(n=N, n_cores=N_CORES, b=B, f=F, k_dim=K):
    """Build the per-core Bass module.  All cores run the same program on
    their own row shard."""
    ns = n // n_cores          # rows per core
    nt_cnt = ns // P           # row tiles per core
    mc_cnt = n // P            # contraction chunks
    ng = mc_cnt // GRP         # adj DMA groups

    nc = bacc.Bacc(None, target_bir_lowering=False, debug=False)

    adjT = nc.dram_tensor("adjT", [k_dim, n, ns], dt.float32, kind="ExternalInput")
    xh = nc.dram_tensor("xh", [P, mc_cnt, b, f], dt.float32, kind="ExternalInput")
    x0h = nc.dram_tensor("x0h", [P, nt_cnt, b, f], dt.float32, kind="ExternalInput")
    xTr = nc.dram_tensor("xTr", [b, f, ns], dt.float32, kind="ExternalInput")
    alpha = nc.dram_tensor("alpha", [ns], dt.float32, kind="ExternalInput")
    beta = nc.dram_tensor("beta", [ns], dt.float32, kind="ExternalInput")
    wT = nc.dram_tensor("wT", [f, f], dt.float32, kind="ExternalInput")
    d = nc.dram_tensor("d", [f], dt.float32, kind="ExternalInput")
    conv_w = nc.dram_tensor("conv_w", [k_dim], dt.float32, kind="ExternalInput")
    conv_b = nc.dram_tensor("conv_b", [1], dt.float32, kind="ExternalInput")
    y = nc.dram_tensor("y", [P, nt_cnt, b, f], dt.bfloat16, kind="ExternalOutput")

    bf = b * f  # stacked batch-feature columns

    with tile.TileContext(nc) as tc, ExitStack() as ctx:
        const = ctx.enter_context(tc.tile_pool(name="const", bufs=1))
        xres = ctx.enter_context(tc.tile_pool(name="xres", bufs=1))
        adj_pool = ctx.enter_context(tc.tile_pool(name="adjp", bufs=3))
        ae_pool = ctx.enter_context(tc.tile_pool(name="aep", bufs=2))
        outp = ctx.enter_context(tc.tile_pool(name="outp", bufs=2))
        psy = ctx.enter_context(tc.tile_pool(name="psy", bufs=1, space="PSUM"))
        psw = ctx.enter_context(tc.tile_pool(name="psw", bufs=1, space="PSUM"))

        # ---------------- constants / gates ----------------
        ident_f = const.tile([f, f], dt.float32, tag="ident_f")
        make_identity(nc, ident_f[:])

        wT_sb = const.tile([f, f], dt.float32, tag="wT_sb")
        nc.gpsimd.dma_start(out=wT_sb[:], in_=wT[:, :])
        d_sb = const.tile([f, 1], dt.float32, tag="d_sb")
        nc.gpsimd.dma_start(out=d_sb[:], in_=d[:, None])
        cw_sb = const.tile([P, k_dim], dt.float32, tag="cw_sb")
        nc.gpsimd.dma_start(out=cw_sb[:], in_=conv_w[None, :].to_broadcast((P, k_dim)))
        cb_sb = const.tile([P, 1], dt.float32, tag="cb_sb")
        nc.gpsimd.dma_start(out=cb_sb[:], in_=conv_b[None, :].to_broadcast((P, 1)))

        al_sb = const.tile([P, nt_cnt], dt.float32, tag="al_sb")
        nc.gpsimd.dma_start(out=al_sb[:], in_=alpha.rearrange("(t p) -> p t", p=P))
        be_sb = const.tile([P, nt_cnt], dt.float32, tag="be_sb")
        nc.gpsimd.dma_start(out=be_sb[:], in_=beta.rearrange("(t p) -> p t", p=P))

        siga = const.tile([P, nt_cnt], dt.float32, tag="siga")
        nc.scalar.activation(siga[:], al_sb[:], AF.Sigmoid)
        sigb = const.tile([P, nt_cnt], dt.float32, tag="sigb")
        nc.scalar.activation(sigb[:], be_sb[:], AF.Sigmoid)
        # svec[p, nt] = 0.5 * sigmoid(alpha)  (row scale for the adj term)
        svec = const.tile([P, nt_cnt], dt.float32, tag="svec")
        nc.vector.tensor_scalar(svec[:], siga[:], 0.5, None, ALU.mult)
        # bias_cb[p, nt] = 0.5 * sigmoid(alpha) * conv_b
        bias_cb = const.tile([P, nt_cnt], dt.float32, tag="bias_cb")
        nc.vector.tensor_scalar(bias_cb[:], svec[:], cb_sb[:, 0:1], None, ALU.mult)

        # ---------------- resident x (bf16) via gpsimd SWDGE cast ----------
        xtr_sb = xres.tile([f, b, ns], dt.float32, tag="xtr_sb")
        nc.gpsimd.dma_start(out=xtr_sb[:], in_=xTr.rearrange("b f i -> f b i"))
        x4 = xres.tile([P, mc_cnt, b, f], dt.bfloat16, tag="x4")
        xchunk = mc_cnt // 8
        for j in range(8):
            nc.gpsimd.dma_start(
                out=x4[:, j * xchunk : (j + 1) * xchunk],
                in_=xh[:, j * xchunk : (j + 1) * xchunk],
            )
        x0_sb = xres.tile([P, nt_cnt, b, f], dt.bfloat16, tag="x0_sb")
        nc.gpsimd.dma_start(out=x0_sb[:], in_=x0h[:, :])
        x0w_sb = xres.tile([P, nt_cnt, bf], dt.float32, tag="x0w_sb")

        # ---------------- psum accumulators: two row-tiles per bank ----------
        n_banks = (nt_cnt + 1) // 2
        psum_y = [
            psy.tile([P, 2 * bf], dt.float32, tag=f"y{i}", name=f"psum_y{i}")
            for i in range(n_banks)
        ]
        psum_w = [
            psw.tile([P, 2 * bf], dt.float32, tag=f"w{i}", name=f"psum_w{i}")
            for i in range(n_banks)
        ]

        def y_region(ntt):
            return psum_y[ntt // 2][:, (ntt % 2) * bf : (ntt % 2 + 1) * bf]

        def w_region(ntt):
            return psum_w[ntt // 2][:, (ntt % 2) * bf : (ntt % 2 + 1) * bf]

        # ---------------- W' = (w * clip(d,0,1)) @ w.T - 2I ----------------
        dc = const.tile([f, 1], dt.float32, tag="dc")
        nc.vector.tensor_scalar(dc[:], d_sb[:], 0.0, 1.0, ALU.max, ALU.min)
        # wdcT[j, f1] = dc[j] * w[f1, j]  (per-partition scale of w^T)
        wdcT = const.tile([f, f], dt.float32, tag="wdcT")
        nc.vector.tensor_scalar(wdcT[:], wT_sb[:], dc[:], None, ALU.mult)
        pw = psum_w[0][:f, 0:f]  # scratch before the xw accumulation opens
        nc.tensor.matmul(pw, wdcT[:], wT_sb[:], start=True, stop=True)
        wp = const.tile([f, f], dt.float32, tag="wp")
        nc.vector.scalar_tensor_tensor(
            wp[:], ident_f[:], -2.0, pw, ALU.mult, ALU.add
        )

        # ---------------- xw = x_rows @ (W - 2I) into psum_w ----------------
        for ntt in range(nt_cnt):
            for bb in range(b):
                nc.tensor.matmul(
                    w_region(ntt)[:, bb * f : (bb + 1) * f],
                    xtr_sb[:, bb, ntt * P : (ntt + 1) * P],
                    wp[:],
                    start=True,
                    stop=True,
                    skip_group_check=True,
                )

        # ---------------- main loop: stream adjT, combine, matmul ------------
        # Combine a_eff = cw0*a0 + cw1*a1 on DVE (in-place, two ops).  The
        # Scalar/Sync engines carry ONLY DMA triggers in the main loop, or
        # the HWDGE sequencer blocks on compute and the queue runs dry.
        # Group granularity amortizes per-op overhead; the last group is done
        # per-chunk to shorten the post-last-byte tail.
        for g in range(ng):
            last = g == ng - 1
            a0 = adj_pool.tile([P, GRP, ns], dt.float32, tag="a0", name="adj_t0")
            a1 = adj_pool.tile([P, GRP, ns], dt.float32, tag="a1", name="adj_t1")
            # The last group's DMAs are issued per-chunk so its first chunks
            # land (and start computing) before the final bytes arrive.
            dsub = range(GRP) if last else [slice(0, GRP)]
            for dd in dsub:
                dsl = dd if isinstance(dd, slice) else slice(dd, dd + 1)
                rows = slice((g * GRP + dsl.start) * P, (g * GRP + dsl.stop) * P)
                nc.sync.dma_start(
                    out=a0[:, dsl],
                    in_=adjT[0, rows, :].rearrange("(c p) i -> p c i", p=P),
                )
                nc.scalar.dma_start(
                    out=a1[:, dsl],
                    in_=adjT[1, rows, :].rearrange("(c p) i -> p c i", p=P),
                )
            if g == ng // 2:
                # x0*sigmoid(beta) + xw folded into one SBUF tile mid-kernel
                # (x0 and the xw psums are long since ready; DVE has slack
                # here) so the tail epilogue is a single op + tanh per tile.
                for ntt in range(nt_cnt):
                    nc.vector.scalar_tensor_tensor(
                        x0w_sb[:, ntt],
                        x0_sb[:, ntt].rearrange("p b f -> p (b f)"),
                        sigb[:, ntt : ntt + 1],
                        w_region(ntt),
                        ALU.mult,
                        ALU.add,
                    )
            ae = ae_pool.tile([P, GRP, ns], dt.bfloat16, tag="ae")
            sub = range(GRP) if last else [slice(0, GRP)]
            for cc in sub:
                csl = cc if isinstance(cc, slice) else slice(cc, cc + 1)
                nc.vector.tensor_scalar(
                    ae[:, csl], a0[:, csl], cw_sb[:, 0:1], None, ALU.mult
                )
                nc.vector.scalar_tensor_tensor(
                    ae[:, csl], a1[:, csl], cw_sb[:, 1:2], ae[:, csl],
                    ALU.mult, ALU.add,
                )
                for c in range(csl.start, csl.stop):
                    mc = g * GRP + c
                    for ntt in range(nt_cnt):
                        nc.tensor.matmul(
                            y_region(ntt),
                            ae[:, c, ntt * P : (ntt + 1) * P],
                            x4[:, mc, :, :],
                            start=(mc == 0),
                            stop=(mc == mc_cnt - 1),
                            skip_group_check=True,
                        )

        # ---------------- epilogue: tanh(svec*psum_y + x0w + bias) ----------
        # y writes go on the HWDGE queues, which are idle once the adj
        # stream finishes.
        for ntt in range(nt_cnt):
            t2 = outp.tile([P, bf], dt.float32, tag="t2")
            nc.vector.scalar_tensor_tensor(
                t2[:], y_region(ntt), svec[:, ntt : ntt + 1], x0w_sb[:, ntt],
                ALU.mult, ALU.add,
            )
            ot = outp.tile([P, bf], dt.bfloat16, tag="ot")
            nc.scalar.activation(ot[:], t2[:], AF.Tanh, bias=bias_cb[:, ntt : ntt + 1])
            eng = nc.sync if ntt % 2 == 0 else nc.scalar
            eng.dma_start(
                out=y[:, ntt], in_=ot[:].rearrange("p (b f) -> p b f", b=b)
            )

    nc.finalize()
    return nc


_NC_CACHE = {}


def _get_nc(key=(N, N_CORES, B, F, K)):
    if key not in _NC_CACHE:
        _NC_CACHE[key] = build_kernel(*key)
    return _NC_CACHE[key]


def make_in_maps(x, x0, adj, alpha, beta, w, d, conv_w, conv_b, n_cores=N_CORES):
    """Host-side staging: slice per-core row shards and pre-permute layouts
    so every device DMA reads large contiguous per-partition chunks."""
    b, n, f = x.shape
    ns = n // n_cores
    f32 = np.float32
    xh = np.ascontiguousarray(
        x.reshape(b, n // P, P, f).transpose(2, 1, 0, 3), dtype=f32
    )
    wTh = np.ascontiguousarray(w.T, dtype=f32)
    in_maps = []
    for c in range(n_cores):
        rows = slice(c * ns, (c + 1) * ns)
        in_maps.append(
            {
                "adjT": np.ascontiguousarray(
                    adj[:, rows, :].transpose(0, 2, 1), dtype=f32
                ),
                "xh": xh,
                "x0h": np.ascontiguousarray(
                    x0[:, rows, :].reshape(b, ns // P, P, f).transpose(2, 1, 0, 3),
                    dtype=f32,
                ),
                "xTr": np.ascontiguousarray(
                    x[:, rows, :].transpose(0, 2, 1), dtype=f32
                ),
                "alpha": np.ascontiguousarray(alpha[rows], dtype=f32),
                "beta": np.ascontiguousarray(beta[rows], dtype=f32),
                "wT": wTh,
                "d": np.ascontiguousarray(d, dtype=f32),
                "conv_w": np.ascontiguousarray(conv_w, dtype=f32),
                "conv_b": np.ascontiguousarray(conv_b, dtype=f32),
            }
        )
    return in_maps


def kernel(x, x0, adj, alpha, beta, w, d, conv_w, conv_b):
    x = np.asarray(x)
    x0 = np.asarray(x0)
    adj = np.asarray(adj)
    alpha = np.asarray(alpha)
    beta = np.asarray(beta)
    w = np.asarray(w)
    d = np.asarray(d)
    conv_w = np.asarray(conv_w)
    conv_b = np.asarray(conv_b)

    nc = _get_nc()
    in_maps = make_in_maps(x, x0, adj, alpha, beta, w, d, conv_w, conv_b)
    res = run_bass_kernel_spmd(nc, in_maps, core_ids=list(range(N_CORES)))
    b, n, f = x.shape
    ns = n // N_CORES
    parts = []
    for c in range(N_CORES):
        yc = res.results[c]["y"]  # [P, nt, b, f]
        parts.append(yc.transpose(2, 1, 0, 3).reshape(b, ns, f))
    out = np.concatenate(parts, axis=1)
    return out.astype(np.float32)


# revision 32
# speedup vs baseline: 1.0368x; 1.0368x over previous
"""Trainium2 Bass kernel for the GODEFunc graph-ODE message-passing module.

Math (per batch b):
    xa   = sum_k conv_w[k] * (adj[k] @ x[b]) + conv_b
    W    = (w * clip(d,0,1)) @ w.T
    out  = tanh(0.5*sigmoid(alpha) * xa - 2*x[b] + x[b] @ W + x0[b]*sigmoid(beta))

Sharding: rows (nodes) split across 8 cores; each core computes its
1024-row slice of the output for all batches.  No collectives needed.

Key layout decisions (all host-side staging, no host arithmetic):
  - adj is passed per-core TRANSPOSED (adjT[k][m, i] = adj[k, row_i, m]) so
    the PE reads contraction-major lhsT tiles straight from SBUF -- no PE
    transposes, no PSUM->SBUF copies.
  - x / x0 / x^T are passed pre-permuted so every DMA reads >=4KB
    contiguous per partition (the natural layouts produce 128-256B
    descriptors which run far below DMA line rate).
  - adj k=0 streams on the Sync HWDGE queue, k=1 on the Scalar HWDGE
    queue, x-family on the GpSimd SWDGE queue (casting x/x0 to bf16
    in-DMA): three parallel DMA queues; the two HWDGE queues together
    saturate the ~358 GB/s per-core HBM limit.
  - The Sync/Scalar engines carry ONLY DMA triggers during the main loop:
    an HWDGE dma_start waits at the issuing engine's sequencer, so any
    compute op between triggers serializes the queue with compute.

Per-core kernel structure:
  - DVE combines the two fp32 adjT streams into one bf16 a_eff tile
    (conv_w folded in, in-place, group-batched); the 0.5*sigmoid(alpha)
    row scale is applied at the epilogue where rows are the partition dim.
  - Main matmuls accumulate psum_y[ntt] += a_effT_slice.T @ x4[mc] over
    all 64 contraction chunks (3-group-deep DMA prefetch; the last group's
    DMAs and combines are per-chunk to shorten the post-last-byte tail).
  - x @ (W - 2I) accumulates into a second PSUM set via host-provided x^T
    (fp32: this is the dominant output term, bf16 here costs ~1e-2 error).
  - x0*sig(beta) + psum_xw is folded into one SBUF tile mid-kernel, so the
    tail epilogue is one DVE op + tanh per row tile; y writes go out on
    the by-then-idle HWDGE queues.
"""

import sys

for _p in ("/opt/trn_rl_repo",):
    if _p not in sys.path:
        sys.path.insert(0, _p)

from contextlib import ExitStack

import numpy as np

import concourse.bass as bass
import concourse.mybir as mybir
import concourse.tile as tile
from concourse import bacc
from concourse.bass_utils import run_bass_kernel_spmd
from concourse.masks import make_identity

dt = mybir.dt
AF = mybir.ActivationFunctionType
ALU = mybir.AluOpType

B, N, F, K = 4, 8192, 64, 2
N_CORES = 8
P = 128
GRP = 2  # contraction chunks per adj DMA group


def build_kernel(n=N, n_cores=N_CORES, b=B, f=F, k_dim=K):
    """Build the per-core Bass module.  All cores run the same program on
    their own row shard."""
    ns = n // n_cores          # rows per core
    nt_cnt = ns // P           # row tiles per core
    mc_cnt = n // P            # contraction chunks
    ng = mc_cnt // GRP         # adj DMA groups

    nc = bacc.Bacc(None, target_bir_lowering=False, debug=False)

    adjT = nc.dram_tensor("adjT", [k_dim, n, ns], dt.float32, kind="ExternalInput")
    xh = nc.dram_tensor("xh", [P, mc_cnt, b, f], dt.float32, kind="ExternalInput")
    x0h = nc.dram_tensor("x0h", [P, nt_cnt, b, f], dt.float32, kind="ExternalInput")
    xTr = nc.dram_tensor("xTr", [b, f, ns], dt.float32, kind="ExternalInput")
    alpha = nc.dram_tensor("alpha", [ns], dt.float32, kind="ExternalInput")
    beta = nc.dram_tensor("beta", [ns], dt.float32, kind="ExternalInput")
    wT = nc.dram_tensor("wT", [f, f], dt.float32, kind="ExternalInput")
    d = nc.dram_tensor("d", [f], dt.float32, kind="ExternalInput")
    conv_w = nc.dram_tensor("conv_w", [k_dim], dt.float32, kind="ExternalInput")
    conv_b = nc.dram_tensor("conv_b", [1], dt.float32, kind="ExternalInput")
    y = nc.dram_tensor("y", [P, nt_cnt, b, f], dt.bfloat16, kind="ExternalOutput")

    bf = b * f  # stacked batch-feature columns

    with tile.TileContext(nc) as tc, ExitStack() as ctx:
        const = ctx.enter_context(tc.tile_pool(name="const", bufs=1))
        xres = ctx.enter_context(tc.tile_pool(name="xres", bufs=1))
        adj_pool = ctx.enter_context(tc.tile_pool(name="adjp", bufs=6))
        ae_pool = ctx.enter_context(tc.tile_pool(name="aep", bufs=2))
        outp = ctx.enter_context(tc.tile_pool(name="outp", bufs=2))
        psy = ctx.enter_context(tc.tile_pool(name="psy", bufs=1, space="PSUM"))
        psw = ctx.enter_context(tc.tile_pool(name="psw", bufs=1, space="PSUM"))

        # ---------------- constants / gates ----------------
        ident_f = const.tile([f, f], dt.float32, tag="ident_f")
        make_identity(nc, ident_f[:])

        wT_sb = const.tile([f, f], dt.float32, tag="wT_sb")
        nc.gpsimd.dma_start(out=wT_sb[:], in_=wT[:, :])
        d_sb = const.tile([f, 1], dt.float32, tag="d_sb")
        nc.gpsimd.dma_start(out=d_sb[:], in_=d[:, None])
        cw_sb = const.tile([P, k_dim], dt.float32, tag="cw_sb")
        nc.gpsimd.dma_start(out=cw_sb[:], in_=conv_w[None, :].to_broadcast((P, k_dim)))
        cb_sb = const.tile([P, 1], dt.float32, tag="cb_sb")
        nc.gpsimd.dma_start(out=cb_sb[:], in_=conv_b[None, :].to_broadcast((P, 1)))

        al_sb = const.tile([P, nt_cnt], dt.float32, tag="al_sb")
        nc.gpsimd.dma_start(out=al_sb[:], in_=alpha.rearrange("(t p) -> p t", p=P))
        be_sb = const.tile([P, nt_cnt], dt.float32, tag="be_sb")
        nc.gpsimd.dma_start(out=be_sb[:], in_=beta.rearrange("(t p) -> p t", p=P))

        siga = const.tile([P, nt_cnt], dt.float32, tag="siga")
        nc.scalar.activation(siga[:], al_sb[:], AF.Sigmoid)
        sigb = const.tile([P, nt_cnt], dt.float32, tag="sigb")
        nc.scalar.activation(sigb[:], be_sb[:], AF.Sigmoid)
        # svec[p, nt] = 0.5 * sigmoid(alpha)  (row scale for the adj term)
        svec = const.tile([P, nt_cnt], dt.float32, tag="svec")
        nc.vector.tensor_scalar(svec[:], siga[:], 0.5, None, ALU.mult)
        # bias_cb[p, nt] = 0.5 * sigmoid(alpha) * conv_b
        bias_cb = const.tile([P, nt_cnt], dt.float32, tag="bias_cb")
        nc.vector.tensor_scalar(bias_cb[:], svec[:], cb_sb[:, 0:1], None, ALU.mult)

        # ---------------- resident x (bf16) via gpsimd SWDGE cast ----------
        xtr_sb = xres.tile([f, b, ns], dt.float32, tag="xtr_sb")
        nc.gpsimd.dma_start(out=xtr_sb[:], in_=xTr.rearrange("b f i -> f b i"))
        x4 = xres.tile([P, mc_cnt, b, f], dt.bfloat16, tag="x4")
        xchunk = mc_cnt // 8
        for j in range(8):
            nc.gpsimd.dma_start(
                out=x4[:, j * xchunk : (j + 1) * xchunk],
                in_=xh[:, j * xchunk : (j + 1) * xchunk],
            )
        x0_sb = xres.tile([P, nt_cnt, b, f], dt.bfloat16, tag="x0_sb")
        nc.gpsimd.dma_start(out=x0_sb[:], in_=x0h[:, :])
        x0w_sb = xres.tile([P, nt_cnt, bf], dt.float32, tag="x0w_sb")

        # ---------------- psum accumulators: two row-tiles per bank ----------
        n_banks = (nt_cnt + 1) // 2
        psum_y = [
            psy.tile([P, 2 * bf], dt.float32, tag=f"y{i}", name=f"psum_y{i}")
            for i in range(n_banks)
        ]
        psum_w = [
            psw.tile([P, 2 * bf], dt.float32, tag=f"w{i}", name=f"psum_w{i}")
            for i in range(n_banks)
        ]

        def y_region(ntt):
            return psum_y[ntt // 2][:, (ntt % 2) * bf : (ntt % 2 + 1) * bf]

        def w_region(ntt):
            return psum_w[ntt // 2][:, (ntt % 2) * bf : (ntt % 2 + 1) * bf]

        # ---------------- W' = (w * clip(d,0,1)) @ w.T - 2I ----------------
        dc = const.tile([f, 1], dt.float32, tag="dc")
        nc.vector.tensor_scalar(dc[:], d_sb[:], 0.0, 1.0, ALU.max, ALU.min)
        # wdcT[j, f1] = dc[j] * w[f1, j]  (per-partition scale of w^T)
        wdcT = const.tile([f, f], dt.float32, tag="wdcT")
        nc.vector.tensor_scalar(wdcT[:], wT_sb[:], dc[:], None, ALU.mult)
        pw = psum_w[0][:f, 0:f]  # scratch before the xw accumulation opens
        nc.tensor.matmul(pw, wdcT[:], wT_sb[:], start=True, stop=True)
        wp = const.tile([f, f], dt.float32, tag="wp")
        nc.vector.scalar_tensor_tensor(
            wp[:], ident_f[:], -2.0, pw, ALU.mult, ALU.add
        )

        # ---------------- xw = x_rows @ (W - 2I) into psum_w ----------------
        for ntt in range(nt_cnt):
            for bb in range(b):
                nc.tensor.matmul(
                    w_region(ntt)[:, bb * f : (bb + 1) * f],
                    xtr_sb[:, bb, ntt * P : (ntt + 1) * P],
                    wp[:],
                    start=True,
                    stop=True,
                    skip_group_check=True,
                )

        # ---------------- main loop: stream adjT, combine, matmul ------------
        # Combine a_eff = cw0*a0 + cw1*a1 on DVE (in-place, two ops).  The
        # Scalar/Sync engines carry ONLY DMA triggers in the main loop, or
        # the HWDGE sequencer blocks on compute and the queue runs dry.
        # Group granularity amortizes per-op overhead; the last group is done
        # per-chunk to shorten the post-last-byte tail.
        for g in range(ng):
            last = g == ng - 1
            a0 = adj_pool.tile([P, GRP, ns], dt.float32, tag="a0", name="adj_t0")
            a1 = adj_pool.tile([P, GRP, ns], dt.float32, tag="a1", name="adj_t1")
            # The last group's DMAs are issued per-chunk so its first chunks
            # land (and start computing) before the final bytes arrive.
            dsub = range(GRP) if last else [slice(0, GRP)]
            for dd in dsub:
                dsl = dd if isinstance(dd, slice) else slice(dd, dd + 1)
                rows = slice((g * GRP + dsl.start) * P, (g * GRP + dsl.stop) * P)
                nc.sync.dma_start(
                    out=a0[:, dsl],
                    in_=adjT[0, rows, :].rearrange("(c p) i -> p c i", p=P),
                )
                nc.scalar.dma_start(
                    out=a1[:, dsl],
                    in_=adjT[1, rows, :].rearrange("(c p) i -> p c i", p=P),
                )
            if g == ng // 2:
                # x0*sigmoid(beta) + xw folded into one SBUF tile mid-kernel
                # (x0 and the xw psums are long since ready; DVE has slack
                # here) so the tail epilogue is a single op + tanh per tile.
                for ntt in range(nt_cnt):
                    nc.vector.scalar_tensor_tensor(
                        x0w_sb[:, ntt],
                        x0_sb[:, ntt].rearrange("p b f -> p (b f)"),
                        sigb[:, ntt : ntt + 1],
                        w_region(ntt),
                        ALU.mult,
                        ALU.add,
                    )
            ae = ae_pool.tile([P, GRP, ns], dt.bfloat16, tag="ae")
            sub = range(GRP) if last else [slice(0, GRP)]
            for cc in sub:
                csl = cc if isinstance(cc, slice) else slice(cc, cc + 1)
                nc.vector.tensor_scalar(
                    ae[:, csl], a0[:, csl], cw_sb[:, 0:1], None, ALU.mult
                )
                nc.vector.scalar_tensor_tensor(
                    ae[:, csl], a1[:, csl], cw_sb[:, 1:2], ae[:, csl],
                    ALU.mult, ALU.add,
                )
                for c in range(csl.start, csl.stop):
                    mc = g * GRP + c
                    for ntt in range(nt_cnt):
                        nc.tensor.matmul(
                            y_region(ntt),
                            ae[:, c, ntt * P : (ntt + 1) * P],
                            x4[:, mc, :, :],
                            start=(mc == 0),
                            stop=(mc == mc_cnt - 1),
                            skip_group_check=True,
                        )

        # ---------------- epilogue: tanh(svec*psum_y + x0w + bias) ----------
        # y writes go on the HWDGE queues, which are idle once the adj
        # stream finishes.
        for ntt in range(nt_cnt):
            t2 = outp.tile([P, bf], dt.float32, tag="t2")
            nc.vector.scalar_tensor_tensor(
                t2[:], y_region(ntt), svec[:, ntt : ntt + 1], x0w_sb[:, ntt],
                ALU.mult, ALU.add,
            )
            ot = outp.tile([P, bf], dt.bfloat16, tag="ot")
            nc.scalar.activation(ot[:], t2[:], AF.Tanh, bias=bias_cb[:, ntt : ntt + 1])
            eng = nc.sync if ntt % 2 == 0 else nc.scalar
            eng.dma_start(
                out=y[:, ntt], in_=ot[:].rearrange("p (b f) -> p b f", b=b)
            )

    nc.finalize()
    return nc


_NC_CACHE = {}


def _get_nc(key=(N, N_CORES, B, F, K)):
    if key not in _NC_CACHE:
        _NC_CACHE[key] = build_kernel(*key)
    return _NC_CACHE[key]


def make_in_maps(x, x0, adj, alpha, beta, w, d, conv_w, conv_b, n_cores=N_CORES):
    """Host-side staging: slice per-core row shards and pre-permute layouts
    so every device DMA reads large contiguous per-partition chunks."""
    b, n, f = x.shape
    ns = n // n_cores
    f32 = np.float32
    xh = np.ascontiguousarray(
        x.reshape(b, n // P, P, f).transpose(2, 1, 0, 3), dtype=f32
    )
    wTh = np.ascontiguousarray(w.T, dtype=f32)
    in_maps = []
    for c in range(n_cores):
        rows = slice(c * ns, (c + 1) * ns)
        in_maps.append(
            {
                "adjT": np.ascontiguousarray(
                    adj[:, rows, :].transpose(0, 2, 1), dtype=f32
                ),
                "xh": xh,
                "x0h": np.ascontiguousarray(
                    x0[:, rows, :].reshape(b, ns // P, P, f).transpose(2, 1, 0, 3),
                    dtype=f32,
                ),
                "xTr": np.ascontiguousarray(
                    x[:, rows, :].transpose(0, 2, 1), dtype=f32
                ),
                "alpha": np.ascontiguousarray(alpha[rows], dtype=f32),
                "beta": np.ascontiguousarray(beta[rows], dtype=f32),
                "wT": wTh,
                "d": np.ascontiguousarray(d, dtype=f32),
                "conv_w": np.ascontiguousarray(conv_w, dtype=f32),
                "conv_b": np.ascontiguousarray(conv_b, dtype=f32),
            }
        )
    return in_maps


def kernel(x, x0, adj, alpha, beta, w, d, conv_w, conv_b):
    x = np.asarray(x)
    x0 = np.asarray(x0)
    adj = np.asarray(adj)
    alpha = np.asarray(alpha)
    beta = np.asarray(beta)
    w = np.asarray(w)
    d = np.asarray(d)
    conv_w = np.asarray(conv_w)
    conv_b = np.asarray(conv_b)

    nc = _get_nc()
    in_maps = make_in_maps(x, x0, adj, alpha, beta, w, d, conv_w, conv_b)
    res = run_bass_kernel_spmd(nc, in_maps, core_ids=list(range(N_CORES)))
    b, n, f = x.shape
    ns = n // N_CORES
    parts = []
    for c in range(N_CORES):
        yc = res.results[c]["y"]  # [P, nt, b, f]
        parts.append(yc.transpose(2, 1, 0, 3).reshape(b, ns, f))
    out = np.concatenate(parts, axis=1)
    return out.astype(np.float32)


# revision 33
# speedup vs baseline: 1.0977x; 1.0587x over previous
"""Trainium2 Bass kernel for the GODEFunc graph-ODE message-passing module.

Math (per batch b):
    xa   = sum_k conv_w[k] * (adj[k] @ x[b]) + conv_b
    W    = (w * clip(d,0,1)) @ w.T
    out  = tanh(0.5*sigmoid(alpha) * xa - 2*x[b] + x[b] @ W + x0[b]*sigmoid(beta))

Sharding: rows (nodes) split across 8 cores; each core computes its
1024-row slice of the output for all batches.  No collectives needed.

Key layout decisions (all host-side staging, no host arithmetic):
  - adj is passed per-core TRANSPOSED (adjT[k][m, i] = adj[k, row_i, m]) so
    the PE reads contraction-major lhsT tiles straight from SBUF -- no PE
    transposes, no PSUM->SBUF copies.
  - x / x0 / x^T are passed pre-permuted so every DMA reads >=4KB
    contiguous per partition (the natural layouts produce 128-256B
    descriptors which run far below DMA line rate).
  - adj k=0 streams on the Sync HWDGE queue, k=1 on the Scalar HWDGE
    queue, x-family on the GpSimd SWDGE queue (casting x/x0 to bf16
    in-DMA): three parallel DMA queues; the two HWDGE queues together
    saturate the ~358 GB/s per-core HBM limit.
  - The Sync/Scalar engines carry ONLY DMA triggers during the main loop:
    an HWDGE dma_start waits at the issuing engine's sequencer, so any
    compute op between triggers serializes the queue with compute.

Per-core kernel structure:
  - DVE combines the two fp32 adjT streams into one bf16 a_eff tile
    (conv_w folded in, in-place, group-batched); the 0.5*sigmoid(alpha)
    row scale is applied at the epilogue where rows are the partition dim.
  - Main matmuls accumulate psum_y[ntt] += a_effT_slice.T @ x4[mc] over
    all 64 contraction chunks (3-group-deep DMA prefetch; the last group's
    DMAs and combines are per-chunk to shorten the post-last-byte tail).
  - x @ (W - 2I) accumulates into a second PSUM set via host-provided x^T
    (fp32: this is the dominant output term, bf16 here costs ~1e-2 error).
  - x0*sig(beta) + psum_xw is folded into one SBUF tile mid-kernel, so the
    tail epilogue is one DVE op + tanh per row tile; y writes go out on
    the by-then-idle HWDGE queues.
"""

import sys

for _p in ("/opt/trn_rl_repo",):
    if _p not in sys.path:
        sys.path.insert(0, _p)

from contextlib import ExitStack

import numpy as np

import concourse.bass as bass
import concourse.mybir as mybir
import concourse.tile as tile
from concourse import bacc
from concourse.bass_utils import run_bass_kernel_spmd
from concourse.masks import make_identity

dt = mybir.dt
AF = mybir.ActivationFunctionType
ALU = mybir.AluOpType

B, N, F, K = 4, 8192, 64, 2
N_CORES = 8
P = 128
GRP = 2  # contraction chunks per adj DMA group


def build_kernel(n=N, n_cores=N_CORES, b=B, f=F, k_dim=K):
    """Build the per-core Bass module.  All cores run the same program on
    their own row shard."""
    ns = n // n_cores          # rows per core
    nt_cnt = ns // P           # row tiles per core
    mc_cnt = n // P            # contraction chunks
    ng = mc_cnt // GRP         # adj DMA groups

    nc = bacc.Bacc(None, target_bir_lowering=False, debug=False)

    adjT = nc.dram_tensor("adjT", [k_dim, n, ns], dt.float32, kind="ExternalInput")
    xh = nc.dram_tensor("xh", [P, mc_cnt, b, f], dt.float32, kind="ExternalInput")
    x0h = nc.dram_tensor("x0h", [P, nt_cnt, b, f], dt.float32, kind="ExternalInput")
    xTr = nc.dram_tensor("xTr", [b, f, ns], dt.float32, kind="ExternalInput")
    alpha = nc.dram_tensor("alpha", [ns], dt.float32, kind="ExternalInput")
    beta = nc.dram_tensor("beta", [ns], dt.float32, kind="ExternalInput")
    wT = nc.dram_tensor("wT", [f, f], dt.float32, kind="ExternalInput")
    d = nc.dram_tensor("d", [f], dt.float32, kind="ExternalInput")
    conv_w = nc.dram_tensor("conv_w", [k_dim], dt.float32, kind="ExternalInput")
    conv_b = nc.dram_tensor("conv_b", [1], dt.float32, kind="ExternalInput")
    y = nc.dram_tensor("y", [P, nt_cnt, b, f], dt.bfloat16, kind="ExternalOutput")

    bf = b * f  # stacked batch-feature columns

    with tile.TileContext(nc) as tc, ExitStack() as ctx:
        const = ctx.enter_context(tc.tile_pool(name="const", bufs=1))
        xres = ctx.enter_context(tc.tile_pool(name="xres", bufs=1))
        adj_pool = ctx.enter_context(tc.tile_pool(name="adjp", bufs=6))
        ae_pool = ctx.enter_context(tc.tile_pool(name="aep", bufs=2))
        outp = ctx.enter_context(tc.tile_pool(name="outp", bufs=2))
        psy = ctx.enter_context(tc.tile_pool(name="psy", bufs=1, space="PSUM"))
        psw = ctx.enter_context(tc.tile_pool(name="psw", bufs=1, space="PSUM"))

        # ---------------- constants / gates ----------------
        ident_f = const.tile([f, f], dt.float32, tag="ident_f")
        make_identity(nc, ident_f[:])

        wT_sb = const.tile([f, f], dt.float32, tag="wT_sb")
        nc.gpsimd.dma_start(out=wT_sb[:], in_=wT[:, :])
        d_sb = const.tile([f, 1], dt.float32, tag="d_sb")
        nc.gpsimd.dma_start(out=d_sb[:], in_=d[:, None])
        cw_sb = const.tile([P, k_dim], dt.float32, tag="cw_sb")
        nc.gpsimd.dma_start(out=cw_sb[:], in_=conv_w[None, :].to_broadcast((P, k_dim)))
        cb_sb = const.tile([P, 1], dt.float32, tag="cb_sb")
        nc.gpsimd.dma_start(out=cb_sb[:], in_=conv_b[None, :].to_broadcast((P, 1)))

        al_sb = const.tile([P, nt_cnt], dt.float32, tag="al_sb")
        nc.gpsimd.dma_start(out=al_sb[:], in_=alpha.rearrange("(t p) -> p t", p=P))
        be_sb = const.tile([P, nt_cnt], dt.float32, tag="be_sb")
        nc.gpsimd.dma_start(out=be_sb[:], in_=beta.rearrange("(t p) -> p t", p=P))

        siga = const.tile([P, nt_cnt], dt.float32, tag="siga")
        nc.scalar.activation(siga[:], al_sb[:], AF.Sigmoid)
        sigb = const.tile([P, nt_cnt], dt.float32, tag="sigb")
        nc.scalar.activation(sigb[:], be_sb[:], AF.Sigmoid)
        # svec[p, nt] = 0.5 * sigmoid(alpha)  (row scale for the adj term)
        svec = const.tile([P, nt_cnt], dt.float32, tag="svec")
        nc.vector.tensor_scalar(svec[:], siga[:], 0.5, None, ALU.mult)
        # bias_cb[p, nt] = 0.5 * sigmoid(alpha) * conv_b
        bias_cb = const.tile([P, nt_cnt], dt.float32, tag="bias_cb")
        nc.vector.tensor_scalar(bias_cb[:], svec[:], cb_sb[:, 0:1], None, ALU.mult)

        # ---------------- resident x (bf16) via gpsimd SWDGE cast ----------
        xtr_sb = xres.tile([f, b, ns], dt.float32, tag="xtr_sb")
        nc.gpsimd.dma_start(out=xtr_sb[:], in_=xTr.rearrange("b f i -> f b i"))
        x4 = xres.tile([P, mc_cnt, b, f], dt.bfloat16, tag="x4")
        xchunk = mc_cnt // 8
        for j in range(8):
            nc.gpsimd.dma_start(
                out=x4[:, j * xchunk : (j + 1) * xchunk],
                in_=xh[:, j * xchunk : (j + 1) * xchunk],
            )
        x0_sb = xres.tile([P, nt_cnt, b, f], dt.bfloat16, tag="x0_sb")
        nc.gpsimd.dma_start(out=x0_sb[:], in_=x0h[:, :])
        x0w_sb = xres.tile([P, nt_cnt, bf], dt.float32, tag="x0w_sb")

        # ---------------- psum accumulators: two row-tiles per bank ----------
        n_banks = (nt_cnt + 1) // 2
        psum_y = [
            psy.tile([P, 2 * bf], dt.float32, tag=f"y{i}", name=f"psum_y{i}")
            for i in range(n_banks)
        ]
        psum_w = [
            psw.tile([P, 2 * bf], dt.float32, tag=f"w{i}", name=f"psum_w{i}")
            for i in range(n_banks)
        ]

        def y_region(ntt):
            return psum_y[ntt // 2][:, (ntt % 2) * bf : (ntt % 2 + 1) * bf]

        def w_region(ntt):
            return psum_w[ntt // 2][:, (ntt % 2) * bf : (ntt % 2 + 1) * bf]

        # ---------------- W' = (w * clip(d,0,1)) @ w.T - 2I ----------------
        dc = const.tile([f, 1], dt.float32, tag="dc")
        nc.vector.tensor_scalar(dc[:], d_sb[:], 0.0, 1.0, ALU.max, ALU.min)
        # wdcT[j, f1] = dc[j] * w[f1, j]  (per-partition scale of w^T)
        wdcT = const.tile([f, f], dt.float32, tag="wdcT")
        nc.vector.tensor_scalar(wdcT[:], wT_sb[:], dc[:], None, ALU.mult)
        pw = psum_w[0][:f, 0:f]  # scratch before the xw accumulation opens
        nc.tensor.matmul(pw, wdcT[:], wT_sb[:], start=True, stop=True)
        wp = const.tile([f, f], dt.float32, tag="wp")
        nc.vector.scalar_tensor_tensor(
            wp[:], ident_f[:], -2.0, pw, ALU.mult, ALU.add
        )

        # ---------------- xw = x_rows @ (W - 2I) into psum_w ----------------
        for ntt in range(nt_cnt):
            for bb in range(b):
                nc.tensor.matmul(
                    w_region(ntt)[:, bb * f : (bb + 1) * f],
                    xtr_sb[:, bb, ntt * P : (ntt + 1) * P],
                    wp[:],
                    start=True,
                    stop=True,
                    skip_group_check=True,
                )

        # ---------------- main loop: stream adjT, combine, matmul ------------
        # Combine a_eff = cw0*a0 + cw1*a1 on DVE (in-place, two ops).  The
        # Scalar/Sync engines carry ONLY DMA triggers in the main loop, or
        # the HWDGE sequencer blocks on compute and the queue runs dry.
        # Group granularity amortizes per-op overhead; the last group is done
        # per-chunk to shorten the post-last-byte tail.
        for g in range(ng):
            last = g >= ng - 2
            a0 = adj_pool.tile([P, GRP, ns], dt.float32, tag="a0", name="adj_t0")
            a1 = adj_pool.tile([P, GRP, ns], dt.float32, tag="a1", name="adj_t1")
            # The last group's DMAs are issued per-chunk so its first chunks
            # land (and start computing) before the final bytes arrive.
            dsub = range(GRP) if last else [slice(0, GRP)]
            for dd in dsub:
                dsl = dd if isinstance(dd, slice) else slice(dd, dd + 1)
                rows = slice((g * GRP + dsl.start) * P, (g * GRP + dsl.stop) * P)
                nc.sync.dma_start(
                    out=a0[:, dsl],
                    in_=adjT[0, rows, :].rearrange("(c p) i -> p c i", p=P),
                )
                nc.scalar.dma_start(
                    out=a1[:, dsl],
                    in_=adjT[1, rows, :].rearrange("(c p) i -> p c i", p=P),
                )
            if g == ng // 2:
                # x0*sigmoid(beta) + xw folded into one SBUF tile mid-kernel
                # (x0 and the xw psums are long since ready; DVE has slack
                # here) so the tail epilogue is a single op + tanh per tile.
                for ntt in range(nt_cnt):
                    nc.vector.scalar_tensor_tensor(
                        x0w_sb[:, ntt],
                        x0_sb[:, ntt].rearrange("p b f -> p (b f)"),
                        sigb[:, ntt : ntt + 1],
                        w_region(ntt),
                        ALU.mult,
                        ALU.add,
                    )
            ae = ae_pool.tile([P, GRP, ns], dt.bfloat16, tag="ae")
            sub = range(GRP) if last else [slice(0, GRP)]
            for cc in sub:
                csl = cc if isinstance(cc, slice) else slice(cc, cc + 1)
                nc.vector.tensor_scalar(
                    ae[:, csl], a0[:, csl], cw_sb[:, 0:1], None, ALU.mult
                )
                nc.vector.scalar_tensor_tensor(
                    ae[:, csl], a1[:, csl], cw_sb[:, 1:2], ae[:, csl],
                    ALU.mult, ALU.add,
                )
                for c in range(csl.start, csl.stop):
                    mc = g * GRP + c
                    for ntt in range(nt_cnt):
                        nc.tensor.matmul(
                            y_region(ntt),
                            ae[:, c, ntt * P : (ntt + 1) * P],
                            x4[:, mc, :, :],
                            start=(mc == 0),
                            stop=(mc == mc_cnt - 1),
                            skip_group_check=True,
                        )

        # ---------------- epilogue: tanh(svec*psum_y + x0w + bias) ----------
        # y writes go on the HWDGE queues, which are idle once the adj
        # stream finishes.
        for ntt in range(nt_cnt):
            t2 = outp.tile([P, bf], dt.float32, tag="t2")
            nc.vector.scalar_tensor_tensor(
                t2[:], y_region(ntt), svec[:, ntt : ntt + 1], x0w_sb[:, ntt],
                ALU.mult, ALU.add,
            )
            ot = outp.tile([P, bf], dt.bfloat16, tag="ot")
            nc.scalar.activation(ot[:], t2[:], AF.Tanh, bias=bias_cb[:, ntt : ntt + 1])
            eng = nc.sync if ntt % 2 == 0 else nc.scalar
            eng.dma_start(
                out=y[:, ntt], in_=ot[:].rearrange("p (b f) -> p b f", b=b)
            )

    nc.finalize()
    return nc


_NC_CACHE = {}


def _get_nc(key=(N, N_CORES, B, F, K)):
    if key not in _NC_CACHE:
        _NC_CACHE[key] = build_kernel(*key)
    return _NC_CACHE[key]


def make_in_maps(x, x0, adj, alpha, beta, w, d, conv_w, conv_b, n_cores=N_CORES):
    """Host-side staging: slice per-core row shards and pre-permute layouts
    so every device DMA reads large contiguous per-partition chunks."""
    b, n, f = x.shape
    ns = n // n_cores
    f32 = np.float32
    xh = np.ascontiguousarray(
        x.reshape(b, n // P, P, f).transpose(2, 1, 0, 3), dtype=f32
    )
    wTh = np.ascontiguousarray(w.T, dtype=f32)
    in_maps = []
    for c in range(n_cores):
        rows = slice(c * ns, (c + 1) * ns)
        in_maps.append(
            {
                "adjT": np.ascontiguousarray(
                    adj[:, rows, :].transpose(0, 2, 1), dtype=f32
                ),
                "xh": xh,
                "x0h": np.ascontiguousarray(
                    x0[:, rows, :].reshape(b, ns // P, P, f).transpose(2, 1, 0, 3),
                    dtype=f32,
                ),
                "xTr": np.ascontiguousarray(
                    x[:, rows, :].transpose(0, 2, 1), dtype=f32
                ),
                "alpha": np.ascontiguousarray(alpha[rows], dtype=f32),
                "beta": np.ascontiguousarray(beta[rows], dtype=f32),
                "wT": wTh,
                "d": np.ascontiguousarray(d, dtype=f32),
                "conv_w": np.ascontiguousarray(conv_w, dtype=f32),
                "conv_b": np.ascontiguousarray(conv_b, dtype=f32),
            }
        )
    return in_maps


def kernel(x, x0, adj, alpha, beta, w, d, conv_w, conv_b):
    x = np.asarray(x)
    x0 = np.asarray(x0)
    adj = np.asarray(adj)
    alpha = np.asarray(alpha)
    beta = np.asarray(beta)
    w = np.asarray(w)
    d = np.asarray(d)
    conv_w = np.asarray(conv_w)
    conv_b = np.asarray(conv_b)

    nc = _get_nc()
    in_maps = make_in_maps(x, x0, adj, alpha, beta, w, d, conv_w, conv_b)
    res = run_bass_kernel_spmd(nc, in_maps, core_ids=list(range(N_CORES)))
    b, n, f = x.shape
    ns = n // N_CORES
    parts = []
    for c in range(N_CORES):
        yc = res.results[c]["y"]  # [P, nt, b, f]
        parts.append(yc.transpose(2, 1, 0, 3).reshape(b, ns, f))
    out = np.concatenate(parts, axis=1)
    return out.astype(np.float32)
